# revision 79
# baseline (speedup 1.0000x reference)
"""Causal single-head attention (B=2, T=4096, C=1024, D=64) on 8 TRN2 cores.

Sharding: core = 4*b + c owns query tiles {c, c+4, ..., c+28} of batch b
(128 rows each) — a strided assignment that balances causal work exactly:
every core attends 144 (key-tile x query-tile) units instead of the 256 a
contiguous block would need at the worst rank.

One SPMD Bass program; all per-core causal structure is data-driven:
  - x[b] rows are permuted on host to [own 8 tiles | remaining 24 tiles
    sorted ascending]. Key slot s (128 keys) therefore holds, on core c,
    global tile c+4s for s<8 ("own"/diagonal area) or the (s-8)-th
    smallest tile not congruent to c mod 4 ("main" area).
  - slot s is attended against query groups j >= jb(s) in one wide S^T
    matmul (jb = s for own slots, (s-8)//3 for main slots); the first 128
    query columns of each slot are the causal boundary: a compile-time
    tril mask (DVE multiply on P) for own slots (the true diagonal), and
    for main slots a -1e30 bias folded into the contraction via 8 extra
    rows: Q' rows 64-71 are group indicators, K' rows 64-71 carry the
    per-core kb8 bias, so exp() masks those blocks with no extra work.
  - denominator comes free from a ones-column in V' (column 64): kernel
    returns unnormalized [65, 1024] = [PV^T ; rowsum]; host divides.
x is sent pre-transposed and chunk-major ([chunk, p, cc, t], 8KB
contiguous per partition -> 128 DMA descriptors per 1MB chunk); the only
on-device transposes are cheap [64,128] PE transposes of V^T -> V.

HW schedule notes (TRN2, measured via NRT neuron-profile): all x chunks
stream on the SP HWDGE queue in need order (concurrent queues halve the
critical chunks' bandwidth; the ACT queue's ring would backpressure the
exps, so it only carries c7 + the output); attends run as 23 units
(jb<=3 slots single, late narrow slots packed into shared psA tiles) in
a 3-stage S || exp || PV software pipeline; ~5us of dummy matmuls warm
the tensor engine to its 2.4 GHz p-state before the projections.
"""

import numpy as np

B, T, C, D = 2, 4096, 1024, 64
NCORES = 8
TQ = 1024          # queries per core (8 tiles of 128)
NSLOT = 32         # key slots of 128 (8 own + 24 main)
DTYPE_NAME = "bfloat16"

_CACHE = {}


def _dtypes():
    import concourse.mybir as mybir
    if DTYPE_NAME == "bfloat16":
        import ml_dtypes
        return mybir.dt.bfloat16, ml_dtypes.bfloat16
    return mybir.dt.float32, np.float32


def _jb(slot):
    """First query group that attends key slot `slot`."""
    return slot if slot < 8 else (slot - 8) // 3


def _build_program(dt_x):
    import concourse.mybir as mybir
    import concourse.tile as tile
    from concourse import bacc
    from contextlib import ExitStack

    f32 = mybir.dt.float32

    nc = bacc.Bacc(
        "TRN2",
        target_bir_lowering=False,
        debug=False,
        num_devices=NCORES,
    )

    # x arrives chunk-major: [chunk, partition, cc, t] with 8KB contiguous
    # per (chunk, partition) so each chunk DMA needs only 128 descriptors
    # (the HWDGE queues are descriptor-rate limited)
    xT_t = nc.dram_tensor("xT", [8, 128, 8, 512], dt_x, kind="ExternalInput")
    wkv_t = nc.dram_tensor("wkv", [128, 8, 128], dt_x, kind="ExternalInput")
    wq_t = nc.dram_tensor("wq", [128, 8, 64], dt_x, kind="ExternalInput")
    kb8_t = nc.dram_tensor("kb8", [8, T], dt_x, kind="ExternalInput")
    qsel_t = nc.dram_tensor("qsel", [8, TQ], dt_x, kind="ExternalInput")
    out_t = nc.dram_tensor("outT", [65, TQ], f32, kind="ExternalOutput")

    xT = xT_t.ap()
    wkv = wkv_t.ap()
    wq = wq_t.ap()
    kb8 = kb8_t.ap()
    qsel = qsel_t.ap()
    outT = out_t.ap()

    with tile.TileContext(nc) as tc, ExitStack() as ctx:
        const = ctx.enter_context(tc.tile_pool(name="const", bufs=1))
        xpool = ctx.enter_context(tc.tile_pool(name="xpool", bufs=8))
        stage = ctx.enter_context(tc.tile_pool(name="stage", bufs=3))
        ppool = ctx.enter_context(tc.tile_pool(name="ppool", bufs=3))
        psA = ctx.enter_context(tc.tile_pool(name="psA", bufs=2, space="PSUM"))
        psP = ctx.enter_context(tc.tile_pool(name="psP", bufs=2, space="PSUM"))
        psO = ctx.enter_context(tc.tile_pool(name="psO", bufs=1, space="PSUM"))

        # persistent SBUF tensors
        KT = const.tile([72, T], dt_x)   # K^T + 8 boundary-bias rows
        VS = const.tile([128, NSLOT, 65], dt_x)  # V rows + ones col 64
        QT = const.tile([72, TQ], dt_x)  # Q^T + 8 group-indicator rows
        wkv_sb = const.tile([128, 8, 128], dt_x)
        wq_sb = const.tile([128, 8, 64], dt_x)
        tril = const.tile([128, 128], dt_x)    # 1 where q >= k (local)

        # Only wq precedes c0 on the SP queue: the critical chain to the
        # first attend is wq -> c0 -> c1 -> q0/q1; wkv is needed one PE
        # block later (kv0) and rides after c1, while the tiny kb8/qsel
        # (needed at the first S matmul ~15us) go on the ACT queue.
        nc.sync.dma_start(out=wq_sb, in_=wq)
        nc.scalar.dma_start(out=KT[64:72, :], in_=kb8)
        # group-indicator rows of Q': row 64+j is 1 exactly on group j's cols
        nc.scalar.dma_start(out=QT[64:72, :], in_=qsel)
        nc.vector.memset(VS[:, :, 64:65], 1.0)
        nc.vector.memset(tril, 1.0)
        nc.gpsimd.affine_select(
            out=tril,
            in_=tril,
            compare_op=mybir.AluOpType.is_ge,
            fill=0.0,
            base=0,
            pattern=[[1, 128]],
            channel_multiplier=-1,
        )

        # prefetch all 8 chunks as whole-chunk DMAs on the SP queue in
        # strict need order (a concurrent queue steals DMA bandwidth from
        # the critical early chunks; only c7 rides the ACT queue — one
        # chunk fits its ring without backpressuring the exps, and it
        # starts late enough (~11us) not to harm c0/c1)
        xts = {}
        for tci in range(8):
            xt = xpool.tile([128, 8, 512], dt_x, tag="xt")
            eng = nc.scalar if tci == 7 else nc.sync
            eng.dma_start(out=xt, in_=xT[tci])
            xts[tci] = xt
            if tci == 1:
                nc.sync.dma_start(out=wkv_sb, in_=wkv)

        vts = {}

        def pe_warmup():
            """~5us of throwaway matmuls gated only on wq (lands ~5us):
            the tensor engine needs ~3us of continuous work to ramp from
            1.2 to 2.4 GHz, so the real projection chain starts hot."""
            w_ps = psP.tile([64, 512], f32, tag="pj")
            for r in range(10):
                nc.tensor.matmul(
                    w_ps,
                    lhsT=wq_sb[:, 0, :],
                    rhs=wq_sb.rearrange("p a d -> p (a d)"),
                    start=(r == 0),
                    stop=(r == 9),
                )

        def q_mm(tci):
            """Q projection matmuls for own chunk tci (0 or 1)."""
            ts = slice(tci * 512, (tci + 1) * 512)
            xt = xts[tci]
            q_ps = psP.tile([64, 512], f32, tag="pj")
            for cc in range(8):
                nc.tensor.matmul(
                    q_ps,
                    lhsT=wq_sb[:, cc, :],
                    rhs=xt[:, cc, :],
                    start=(cc == 0),
                    stop=(cc == 7),
                )
            nc.vector.tensor_copy(QT[0:64, ts], q_ps)

        def proj_mm(tci):
            """KV projection matmuls for chunk tci."""
            ts = slice(tci * 512, (tci + 1) * 512)
            xt = xts[tci]
            kv_ps = psP.tile([128, 512], f32, tag="pj")
            for cc in range(8):
                nc.tensor.matmul(
                    kv_ps,
                    lhsT=wkv_sb[:, cc, :],
                    rhs=xt[:, cc, :],
                    start=(cc == 0),
                    stop=(cc == 7),
                )
            nc.vector.tensor_copy(KT[0:64, ts], kv_ps[0:64, :])
            vt = stage.tile([64, 512], dt_x, tag="vt")
            nc.vector.tensor_copy(vt, kv_ps[64:128, :])
            vts[tci] = vt

        def v_fixup(tci):
            """Transpose V^T chunk -> VS slots (4 transposes, one bank)."""
            vt = vts.pop(tci)
            vq = psP.tile([128, 4, 64], dt_x, tag="pj")
            for sub in range(4):
                nc.tensor.matmul(
                    vq[:, sub, :],
                    lhsT=vt[:, sub * 128:(sub + 1) * 128],
                    rhs=ident,
                    is_transpose=True,
                    start=(sub == 0),
                    stop=(sub == 3),
                    skip_group_check=True,
                )
            nc.vector.tensor_copy(VS[:, tci * 4:tci * 4 + 4, 0:64], vq)

        def proj_pieces(tci):
            """proj_mm split into small closures to smear across attends."""
            xt = xts[tci]
            ts = slice(tci * 512, (tci + 1) * 512)
            kv_ps = psP.tile([128, 512], f32, tag="pj")

            def mk(cc0):
                def f():
                    for cc in (cc0, cc0 + 1):
                        nc.tensor.matmul(
                            kv_ps,
                            lhsT=wkv_sb[:, cc, :],
                            rhs=xt[:, cc, :],
                            start=(cc == 0),
                            stop=(cc == 7),
                        )
                return f

            def finish():
                nc.vector.tensor_copy(KT[0:64, ts], kv_ps[0:64, :])
                vt = stage.tile([64, 512], dt_x, tag="vt")
                nc.vector.tensor_copy(vt, kv_ps[64:128, :])
                vts[tci] = vt

            return [mk(0), mk(2), mk(4), mk(6), finish,
                    lambda: v_fixup(tci)]

        from concourse.masks import make_identity
        ident = const.tile([64, 64], dt_x)
        make_identity(nc, ident)

        pv = psO.tile([65, TQ], f32)

        # Units pack slots whose jb sum to 8, so every unit's S span is
        # exactly [128, 1024]: slot A (wider) at tile cols [0, wA), slot B
        # at [wA, 1024). One full-width exp covers both. tile_col =
        # global_query_col - off per slot.
        def attend_S_unit(unit):
            s_ps = psA.tile([128, TQ], f32, tag="s")
            offs = []
            base = 0
            for slot in unit:
                jb = _jb(slot)
                w = (8 - jb) * 128
                offs.append(jb * 128 - base)
                ks = slice(slot * 128, (slot + 1) * 128)
                # pieces cut at the PSUM bank boundary (col 512)
                cuts = sorted({base, base + w} | ({512} if base < 512 < base + w else set()))
                for lo, hi in zip(cuts, cuts[1:]):
                    nc.tensor.matmul(
                        s_ps[:, lo:hi],
                        lhsT=KT[:, ks],
                        rhs=QT[:, lo + offs[-1]:hi + offs[-1]],
                        start=True, stop=True,
                    )
                base += w
            return s_ps, offs

        def exp_mask(unit, s_ps, offs):
            """One exp over the unit's written span -> boundary masks."""
            span = sum((8 - _jb(s)) * 128 for s in unit)
            p_sb = ppool.tile([128, TQ], dt_x, tag="p")
            nc.scalar.activation(
                p_sb[:, 0:span], s_ps[:, 0:span],
                mybir.ActivationFunctionType.Exp, scale=float(D) ** -0.5,
            )
            # causal boundary: main slots are already masked via the K'/Q'
            # bias rows (exp of -1e30 is 0); own slots need the true tril
            for slot, off in zip(unit, offs):
                if slot < 8:
                    c0 = _jb(slot) * 128 - off
                    nc.vector.tensor_mul(
                        p_sb[:, c0:c0 + 128], p_sb[:, c0:c0 + 128], tril)
            return p_sb

        def pv_accum(unit, p_sb, offs):
            """PV accumulation in <=2 bank-aligned pieces per slot (fewer
            ldweights/matmuls is a big win on HW). The bank-1 piece goes
            first when jb < 4: it has no boundary-mask dependency, so PE
            starts while DVE applies the mask. PSUM accumulation groups are
            per 2KB bank: start on slot 0 (D_0, the first unit, covers both
            banks); bank 0's last writer is slot 19 (the last jb<=3 slot in
            attend order), bank 1's is slot 31 (in the final unit)."""
            for slot, off in zip(unit, offs):
                jb = _jb(slot)
                q0 = jb * 128
                pieces = ([(512, 1024), (q0, 512)] if jb < 4
                          else [(q0, 1024)])
                for lo, hi in pieces:
                    nc.tensor.matmul(
                        pv[:, lo:hi],
                        lhsT=VS[:, slot, :],
                        rhs=p_sb[:, lo - off:hi - off],
                        start=(slot == 0),
                        stop=(slot == 19 and hi == 512)
                        or (slot == 31 and hi == 1024),
                    )

        # Schedule: Q projection is the critical path to the first attend
        # (S rhs spans all 1024 query cols), so q0/kv0/q1 go first; chunk
        # 1's KV projection and the light D_4..D_7 attends are deferred to
        # fill PE gaps late in the stream when DMA is the limiter.
        pe_warmup()
        q_mm(0)
        q_mm(1)
        proj_mm(0)
        v_fixup(0)

        osb = stage.tile([65, TQ], f32, tag="o")

        def stream_bank(h):
            """Copy + DMA out one PSUM bank of pv once its accumulation group
            closes (bank 0 after slot 19, bank 1 at the end). The SP queue
            is drained of x chunks by then."""
            qs = slice(h * 512, (h + 1) * 512)
            nc.vector.tensor_copy(osb[:, qs], pv[:, qs])
            nc.sync.dma_start(out=outT[:, qs], in_=osb[:, qs])

        pending = []   # [(tci, closure)] proj pieces smeared across attends
        queued = set()

        def queue_chunk(ch):
            if ch in queued or not (1 <= ch <= 7):
                return
            queued.add(ch)
            pending.extend((ch, f) for f in proj_pieces(ch))

        def drain_chunk(ch):
            rest = []
            for tcc, f in pending:
                if tcc == ch:
                    f()
                else:
                    rest.append((tcc, f))
            pending[:] = rest

        # attend units: all jb<=3 slots as singles (strictly sequential
        # chunk needs -> each is attendable as soon as its chunk lands);
        # narrow late slots pack into multi-slot units sharing one psA
        # tile + one exp. Unit [19] is the last bank-0 writer (streams
        # bank 0 early); the final unit closes bank 1.
        UNITS = [[0], [1], [2], [3], [4, 5], [6, 7],
                 [8], [9], [10], [11], [12], [13], [14], [15],
                 [16], [17], [18], [19],
                 [20, 23], [21, 22], [24, 25], [26, 27], [28, 29, 30, 31]]

        # queue projection chunks in DMA-arrival order
        for ch in range(1, 8):
            queue_chunk(ch)

        # 3-stage software pipeline over units: S(n) || exp+mask(n-1) ||
        # PV(n-2) gives each exp a full unit of latency slack before its PV
        # reads P (a pair-unit covers two slots per stage).
        spipe = []  # [(unit, s_ps, offs)]
        ppipe = []  # [(unit, p_sb, offs)]

        def do_exp():
            unit, ps, offs = spipe.pop(0)
            ppipe.append((unit, exp_mask(unit, ps, offs), offs))

        def do_pv():
            unit, p, offs = ppipe.pop(0)
            pv_accum(unit, p, offs)
            if 19 in unit:
                stream_bank(0)

        def advance(unit=None):
            if unit is None:  # drain mode
                if spipe:
                    do_exp()
                if ppipe:
                    do_pv()
                return
            for slot in unit:
                drain_chunk(slot // 4)
            ps, offs = attend_S_unit(unit)
            spipe.append((unit, ps, offs))
            if len(spipe) > 1:
                do_exp()
            if len(ppipe) > 1:
                do_pv()

        for idx, unit in enumerate(UNITS):
            # drain the chunks the NEXT unit needs ahead of its S matmuls
            if idx + 1 < len(UNITS):
                for slot in UNITS[idx + 1]:
                    drain_chunk(slot // 4)
            advance(unit)
            for _ in range(2):
                if pending:
                    tcc, f = pending.pop(0)
                    f()
        while spipe or ppipe:
            advance()
        stream_bank(1)

    nc.compile()
    return nc


def _perm(c):
    """Row permutation for core rank c: own tiles then sorted others."""
    own = [c + 4 * j for j in range(8)]
    rest = [t for t in range(32) if t % 4 != c]
    idx = []
    for t in own + rest:
        idx.extend(range(t * 128, (t + 1) * 128))
    return np.array(idx)


def _prep_inputs(x, Wq, Wk, Wv, np_dt):
    """Per-core input maps."""
    wkv = np.empty((128, 8, 128), dtype=np_dt)
    wkv[:, :, 0:64] = Wk.reshape(8, 128, 64).transpose(1, 0, 2)
    wkv[:, :, 64:128] = Wv.reshape(8, 128, 64).transpose(1, 0, 2)
    wq = np.ascontiguousarray(
        Wq.reshape(8, 128, 64).transpose(1, 0, 2)).astype(np_dt)

    in_maps = []
    for core in range(NCORES):
        b, c = divmod(core, 4)
        xp = x[b][_perm(c), :]
        # chunk-major layout: x8[tci, p, a, t] = xp.T[p + 128a, 512 tci + t]
        xT = np.ascontiguousarray(
            xp.T.reshape(8, 128, 8, 512).transpose(2, 1, 0, 3)).astype(np_dt)
        # kb8[j, k]: -1e30 on main slot i's columns iff that slot's
        # boundary block (group j = i//3) is invisible on this core
        # (v' = i - 3*(i//3) >= c); 0 elsewhere (own-slot diagonals are
        # handled by the on-device tril).
        kb8 = np.zeros((8, T), dtype=np.float32)
        for i in range(24):
            jb, v = i // 3, i - 3 * (i // 3)
            if v >= c:
                kb8[jb, (8 + i) * 128:(9 + i) * 128] = -1e30
        qsel = np.zeros((8, TQ), dtype=np.float32)
        for j in range(8):
            qsel[j, j * 128:(j + 1) * 128] = 1.0
        in_maps.append({
            "xT": xT,
            "wkv": wkv,
            "wq": wq,
            "kb8": kb8.astype(np_dt),
            "qsel": qsel.astype(np_dt),
        })
    return in_maps


def _unscramble(results, out=None):
    """Per-core [65, TQ] outputs -> full [B, T, D] normalized output."""
    if out is None:
        out = np.empty((B, T, D), dtype=np.float32)
    for core in range(NCORES):
        b, c = divmod(core, 4)
        o = np.asarray(results[core]).astype(np.float32)
        qkv = (o[0:64, :] / o[64:65, :]).T  # [1024, 64], group-major
        for j in range(8):
            t = c + 4 * j
            out[b, t * 128:(t + 1) * 128, :] = qkv[j * 128:(j + 1) * 128, :]
    return out


def kernel(x, Wq, Wk, Wv, _trace=False):
    from concourse.bass_utils import run_bass_kernel_spmd

    dt_x, np_dt = _dtypes()

    key = ("prog", str(dt_x))
    if key not in _CACHE:
        _CACHE[key] = _build_program(dt_x)
    nc = _CACHE[key]

    in_maps = _prep_inputs(
        np.asarray(x, np.float32), np.asarray(Wq, np.float32),
        np.asarray(Wk, np.float32), np.asarray(Wv, np.float32), np_dt)

    res = run_bass_kernel_spmd(
        nc, in_maps, core_ids=list(range(NCORES)), trace=_trace)

    out = _unscramble([res.results[core]["outT"] for core in range(NCORES)])
    if _trace:
        return out, res
    return out
